# revision 21
# baseline (speedup 1.0000x reference)
"""ParallelRetention Trainium2 Bass kernel.

Problem (per [b,h] slice, B=2 H=16 S=2048 D=64):
    decay  = omask / sqrt(rowsum(omask))          (per-row rsqrt scale)
    ret    = (q @ k^T) * decay
    denom  = clip(|rowsum(ret)|, 1, inf)
    out    = (ret / denom) @ v

Restructured for the hardware:
    rs[q]    = 1/sqrt(max(msum[q], tiny)),  msum = rowsum(omask)
    augT     = [v | 1]^T @ (scores * omask)^T        # [65, S] per (b,h)
    r[q]     = rs[q] * augT[64, q]
    out[q,:] = augT[0:64, q] * rs[q] / max(|r[q]|, 1)

All per-row normalization collapses onto the [65, S] matmul output; the only
full SxS elementwise work is the scores*omask multiply. Scores are computed
transposed ([k, q] layout) so the second matmul consumes them directly with
v as the stationary operand; omask is transposed on-chip with PE transposes,
and msum comes from a DVE accumulate pass over the natural-layout omask
tiles.

Everything on the matmul paths uses float32r (full-rate fp32 mode, ~13-bit
mantissa, measured 1.6e-4 matmul rel err vs 2.3e-3 for bf16): the row-sum
denominator suffers ~40x cancellation amplification, so bf16 anywhere in the
scores/omask/ret chain costs ~2e-2 output error.

The elementwise multiply is split: some score tiles are multiplied directly
from PSUM by DVE (1x mode), the rest are copied to SBUF by ScalarE and
multiplied by GPSIMD, balancing the three engines.

Sharding: 16 heads / 8 cores = 2 heads per core, both batches on the same
core (omask is per-head, so this halves omask traffic per core). SPMD: one
NEFF, per-core input slices.
"""

import os

import numpy as np

B = 2          # batch (full)
H = 16         # heads (full)
S = 2048
D = 64
N_CORES = 8
HC = H // N_CORES  # heads per core
P = 128
QT = S // P    # q tiles per head
KT = S // P    # k tiles per head
QC = 512       # q chunk (free dim of score tiles)
NCH = S // QC  # chunks
TPC = QC // P  # q tiles per chunk

# Tuning knobs: of the 8 jj score-pair groups per (b,chunk), how many go to
# the GPSIMD multiply path (ACT copy + gpsimd TT); the rest multiply on DVE
# directly from PSUM.
GP_PAIRS = int(os.environ.get("KRN_GP_PAIRS", "3"))
N_WARMUP = int(os.environ.get("KRN_WARMUP", "18"))
ROW_PACK = os.environ.get("KRN_ROW_PACK", "1") == "1"

_NC_CACHE = {}


def _build_nc():
    import concourse.mybir as mybir
    import concourse.tile as tile
    from concourse import bacc
    from concourse.masks import make_identity

    F32R = mybir.dt.float32r
    F32 = mybir.dt.float32
    BF = mybir.dt.bfloat16
    MULT = mybir.AluOpType.mult
    ADD = mybir.AluOpType.add

    nc = bacc.Bacc("TRN2", target_bir_lowering=False, debug=False,
                   num_devices=N_CORES)

    q_d = nc.dram_tensor("q", [B, HC, S, D], F32, kind="ExternalInput")
    k_d = nc.dram_tensor("k", [B, HC, S, D], F32, kind="ExternalInput")
    v_d = nc.dram_tensor("v", [B, HC, S, D], F32, kind="ExternalInput")
    om_d = nc.dram_tensor("omask", [HC, S, S], F32, kind="ExternalInput")
    out_d = nc.dram_tensor("out", [B, HC, S, D], F32, kind="ExternalOutput")

    with tile.TileContext(nc) as tc:
        with (
            tc.tile_pool(name="const", bufs=1) as const_pool,
            tc.tile_pool(name="onat", bufs=3) as onat_pool,
            tc.tile_pool(name="omt", bufs=3) as omt_pool,
            tc.tile_pool(name="qkv", bufs=2) as qkv_pool,
            tc.tile_pool(name="work", bufs=3) as work_pool,
            tc.tile_pool(name="small", bufs=4) as small_pool,
            tc.tile_pool(name="outp", bufs=2) as out_pool,
            tc.tile_pool(name="ps_sc", bufs=2, space="PSUM") as ps_sc,
            tc.tile_pool(name="ps_stage", bufs=2, space="PSUM") as ps_stage,
            tc.tile_pool(name="ps_aug", bufs=2, space="PSUM") as ps_aug,
        ):
            # memset/affine_select can't produce f32r directly (and the BIR
            # verifier requires f32r matmul operands to come from an
            # f32r-rounding producer), so build f32 versions and round-copy.
            ident_f = const_pool.tile([P, P], F32, tag="ident_f")
            make_identity(nc, ident_f)
            ident_r = const_pool.tile([P, P], F32R, tag="ident_r")
            nc.vector.tensor_copy(ident_r, ident_f)
            ones_f = const_pool.tile([P, KT], F32, tag="ones_f")
            nc.vector.memset(ones_f, 1.0)

            # PE warmup: ~18 back-to-back matmuls (~7us cold) so the HAM
            # activity monitor lifts the clock gate (1.2 -> 2.4 GHz) before
            # the real matmul stream starts. Overlaps the initial DMA loads.
            warm_w = const_pool.tile([P, P], BF, tag="warm_w")
            nc.vector.tensor_copy(warm_w, ident_f)
            warm_x = const_pool.tile([P, QC], BF, tag="warm_x")
            nc.vector.memset(warm_x, 1.0)
            if N_WARMUP:
                warm_ps = ps_aug.tile([P, QC], F32, tag="aug")
                for _ in range(N_WARMUP):
                    nc.tensor.matmul(warm_ps, warm_w, warm_x,
                                     start=True, stop=True)
                warm_sink = small_pool.tile([P, 1], F32, tag="warm_sink")
                nc.vector.tensor_copy(warm_sink, warm_ps[:, 0:1])

            def prep_inputs(b, h):
                # q tiles with the d-column block duplicated ([p, t, 128] =
                # [q | q]), and k tiles packed pairwise ([p, jj, 128] =
                # [block 2jj | block 2jj+1]), rounded to f32r during the
                # SWDGE DMA. A single [128,128] PE transpose of each then
                # directly yields: qT duplicated into both partition halves,
                # and kT pairs with block 2jj on partitions 0-63 / block
                # 2jj+1 on 64-127 — the layout that lets the two K=64 score
                # matmuls of a pair run concurrently in disjoint PE
                # row-groups.
                qsrc = q_d[b, h].rearrange("(t p) d -> p t d", p=P)
                qn2 = qkv_pool.tile([P, QT, P], F32R, tag="qn")
                nc.gpsimd.dma_start(out=qn2[:, :, 0:D], in_=qsrc)
                nc.gpsimd.dma_start(out=qn2[:, :, D:2 * D], in_=qsrc)
                kn2 = qkv_pool.tile([P, KT // 2, 2, D], F32R, tag="kn")
                nc.gpsimd.dma_start(
                    out=kn2,
                    in_=k_d[b, h].rearrange(
                        "(jj two p) d -> p jj two d", p=P, two=2))
                va = qkv_pool.tile([P, KT, D + 1], F32R, tag="va")
                nc.gpsimd.dma_start(
                    out=va[:, :, 0:D],
                    in_=v_d[b, h].rearrange("(t p) d -> p t d", p=P))
                nc.vector.tensor_copy(
                    va[:, :, D:D + 1].rearrange("p t o -> p (t o)"), ones_f)

                qT = qkv_pool.tile([P, S], F32R, tag="qT")
                for g in range(4):
                    stg = ps_stage.tile([P, 4, P], F32R, tag="stage")
                    for i in range(4):
                        nc.tensor.transpose(
                            stg[:, i, :], qn2[:, g * 4 + i, :], ident_r)
                    nc.scalar.copy(
                        out=qT[:, g * 4 * P:(g + 1) * 4 * P]
                            .rearrange("d (i c) -> d i c", c=P),
                        in_=stg)
                if ROW_PACK:
                    kT = qkv_pool.tile([P, KT // 2, P], F32R, tag="kT")
                    for g in range(2):
                        stg = ps_stage.tile([P, 4, P], F32R, tag="stage")
                        for i in range(4):
                            nc.tensor.transpose(
                                stg[:, i, :],
                                kn2[:, g * 4 + i, :, :]
                                    .rearrange("p two d -> p (two d)"),
                                ident_r)
                        nc.scalar.copy(
                            out=kT[:, g * 4:(g + 1) * 4, :], in_=stg)
                else:
                    kT = qkv_pool.tile([D, KT, P], F32R, tag="kT")
                    for g in range(4):
                        stg = ps_stage.tile([D, 4, P], F32R, tag="stage")
                        for i in range(4):
                            j = g * 4 + i
                            nc.tensor.transpose(
                                stg[:, i, :],
                                kn2[:, j // 2, j % 2, :], ident_r)
                        nc.scalar.copy(
                            out=kT[:, g * 4:(g + 1) * 4, :], in_=stg)
                return qT, kT, va

            for h in range(HC):
                msum = small_pool.tile([P, QT], F32, tag="msum")
                rs = small_pool.tile([P, QT], F32, tag="rs")

                prepped = [prep_inputs(b, h) for b in range(B)]

                for ch in range(NCH):
                    csl = slice(ch * TPC, (ch + 1) * TPC)
                    # natural omask tiles for this q-chunk (f32r), 2 halves
                    onats = []
                    for half in range(2):
                        onat = onat_pool.tile([P, 2, S], F32R, tag="onat")
                        r0 = ch * QC + half * 2 * P
                        nc.gpsimd.dma_start(
                            out=onat,
                            in_=om_d[h, r0:r0 + 2 * P, :]
                                .rearrange("(t p) k -> p t k", p=P))
                        onats.append(onat)

                    # msum columns for this chunk's q tiles (accumulate pass
                    # split between DVE and ACT to balance load)
                    for t in range(TPC):
                        qt = ch * TPC + t
                        dummy = work_pool.tile([P, S], BF, tag="msum_dummy")
                        src = onats[t // 2][:, t % 2, :]
                        if t % 2 == 0:
                            nc.vector.tensor_scalar(
                                dummy, src, 1.0, 0.0, MULT, ADD,
                                accum_out=msum[:, qt:qt + 1])
                        else:
                            nc.scalar.activation(
                                dummy, src,
                                mybir.ActivationFunctionType.Copy,
                                accum_out=msum[:, qt:qt + 1])

                    # rs = 1/sqrt(max(msum, tiny)) for this chunk
                    nc.vector.tensor_scalar_max(
                        msum[:, csl], msum[:, csl], 1e-30)
                    nc.scalar.sqrt(rs[:, csl], msum[:, csl])
                    nc.vector.reciprocal(rs[:, csl], rs[:, csl])

                    # build omaskT for this chunk: 2 halves of [128, 8, QC]
                    omts = []
                    for half in range(2):
                        omt = omt_pool.tile([P, KT // 2, QC], F32R, tag="omt")
                        for j8 in range(KT // 2):
                            j = half * (KT // 2) + j8
                            stg = ps_stage.tile([P, TPC, P], F32R, tag="stage")
                            for t in range(TPC):
                                nc.tensor.transpose(
                                    stg[:, t, :],
                                    onats[t // 2][:, t % 2, j * P:(j + 1) * P],
                                    ident_r)
                            nc.scalar.copy(
                                out=omt[:, j8, :]
                                    .rearrange("p (t c) -> p t c", c=P),
                                in_=stg)
                        omts.append(omt)

                    def omt_at(jj):
                        half = (jj * 2) // (KT // 2)
                        j8 = (jj * 2) % (KT // 2)
                        return omts[half][:, j8:j8 + 2, :]

                    for b in range(B):
                        qT, kT, va = prepped[b]
                        aug = ps_aug.tile([D + 1, QC], F32, tag="aug")
                        for jj in range(KT // 2):
                            sc = ps_sc.tile([P, 2, QC], F32, tag="scores")
                            for j2 in range(2):
                                base = j2 * D if ROW_PACK else 0
                                lhsT = (kT[base:base + D, jj, :] if ROW_PACK
                                        else kT[:, jj * 2 + j2, :])
                                nc.tensor.matmul(
                                    sc[:, j2, :], lhsT,
                                    qT[base:base + D,
                                       ch * QC:(ch + 1) * QC],
                                    start=True, stop=True)
                            ret = work_pool.tile([P, 2, QC], F32R, tag="ret")
                            if jj >= GP_PAIRS:
                                # DVE multiply directly from PSUM
                                nc.vector.tensor_mul(ret, sc, omt_at(jj))
                            else:
                                # ACT copy to SBUF + GPSIMD multiply
                                scb = work_pool.tile([P, 2, QC], F32,
                                                     tag="scb")
                                nc.scalar.copy(out=scb, in_=sc)
                                nc.gpsimd.tensor_mul(ret, scb, omt_at(jj))
                            for j2 in range(2):
                                j = jj * 2 + j2
                                nc.tensor.matmul(
                                    aug, va[:, j, :], ret[:, j2, :],
                                    start=(j == 0), stop=(j == KT - 1),
                                    skip_group_check=True)

                        # postprocess: [65, QC] -> scaled [q, d] output
                        augs = work_pool.tile([D + 1, QC], F32, tag="augs")
                        nc.scalar.copy(out=augs, in_=aug)
                        autp = ps_stage.tile([P, TPC, D + 1], F32, tag="stage")
                        for t in range(TPC):
                            nc.tensor.transpose(
                                autp[:, t, :], augs[:, t * P:(t + 1) * P],
                                ident_f[0:D + 1, 0:D + 1])
                        scal = small_pool.tile([P, TPC], F32, tag="scal")
                        nc.vector.tensor_mul(
                            scal,
                            autp[:, :, D:D + 1].rearrange("p t o -> p (t o)"),
                            rs[:, csl])
                        nc.scalar.activation(
                            scal, scal, mybir.ActivationFunctionType.Abs)
                        nc.vector.tensor_scalar_max(scal, scal, 1.0)
                        nc.vector.reciprocal(scal, scal)
                        nc.vector.tensor_mul(scal, scal, rs[:, csl])
                        ob = out_pool.tile([P, TPC, D], F32, tag="ob")
                        for t in range(TPC):
                            nc.vector.tensor_scalar(
                                ob[:, t, :], autp[:, t, 0:D],
                                scal[:, t:t + 1], None, MULT)
                        nc.sync.dma_start(
                            out=out_d[b, h, ch * QC:(ch + 1) * QC, :]
                                .rearrange("(t p) d -> p t d", p=P),
                            in_=ob)

    nc.compile()
    return nc


def _get_nc():
    if "nc" not in _NC_CACHE:
        _NC_CACHE["nc"] = _build_nc()
    return _NC_CACHE["nc"]


def kernel(q, k, v, omask, _trace=False):
    from concourse.bass_utils import run_bass_kernel_spmd

    nc = _get_nc()
    in_maps = []
    for c in range(N_CORES):
        hs = slice(c * HC, (c + 1) * HC)
        in_maps.append({
            "q": np.ascontiguousarray(q[:, hs]),
            "k": np.ascontiguousarray(k[:, hs]),
            "v": np.ascontiguousarray(v[:, hs]),
            "omask": np.ascontiguousarray(omask[hs]),
        })
    res = run_bass_kernel_spmd(nc, in_maps, core_ids=list(range(N_CORES)),
                               trace=_trace)
    out = np.concatenate([res.results[c]["out"] for c in range(N_CORES)],
                         axis=1)
    if _trace:
        kernel.last_results = res
    return out


# revision 22
# speedup vs baseline: 1.1882x; 1.1882x over previous
"""ParallelRetention Trainium2 Bass kernel.

Problem (per [b,h] slice, B=2 H=16 S=2048 D=64):
    decay  = omask / sqrt(rowsum(omask))          (per-row rsqrt scale)
    ret    = (q @ k^T) * decay
    denom  = clip(|rowsum(ret)|, 1, inf)
    out    = (ret / denom) @ v

Restructured for the hardware:
    rs[q]    = 1/sqrt(max(msum[q], tiny)),  msum = rowsum(omask)
    augT     = [v | 1]^T @ (scores * omask)^T        # [65, S] per (b,h)
    r[q]     = rs[q] * augT[64, q]
    out[q,:] = augT[0:64, q] * rs[q] / max(|r[q]|, 1)

All per-row normalization collapses onto the [65, S] matmul output; the only
full SxS elementwise work is the scores*omask multiply. Scores are computed
transposed ([k, q] layout) so the second matmul consumes them directly with
v as the stationary operand; omask is transposed on-chip with PE transposes,
and msum comes from a DVE accumulate pass over the natural-layout omask
tiles.

Everything on the matmul paths uses float32r (full-rate fp32 mode, ~13-bit
mantissa, measured 1.6e-4 matmul rel err vs 2.3e-3 for bf16): the row-sum
denominator suffers ~40x cancellation amplification, so bf16 anywhere in the
scores/omask/ret chain costs ~2e-2 output error.

The elementwise multiply is split: some score tiles are multiplied directly
from PSUM by DVE (1x mode), the rest are copied to SBUF by ScalarE and
multiplied by GPSIMD, balancing the three engines.

Sharding: 16 heads / 8 cores = 2 heads per core, both batches on the same
core (omask is per-head, so this halves omask traffic per core). SPMD: one
NEFF, per-core input slices.
"""

import os

import numpy as np

B = 2          # batch (full)
H = 16         # heads (full)
S = 2048
D = 64
N_CORES = 8
HC = H // N_CORES  # heads per core
P = 128
QT = S // P    # q tiles per head
KT = S // P    # k tiles per head
QC = 512       # q chunk (free dim of score tiles)
NCH = S // QC  # chunks
TPC = QC // P  # q tiles per chunk

# Tuning knobs: of the 8 jj score-pair groups per (b,chunk), how many go to
# the GPSIMD multiply path (ACT copy + gpsimd TT); the rest multiply on DVE
# directly from PSUM.
GP_PAIRS = int(os.environ.get("KRN_GP_PAIRS", "3"))
N_WARMUP = int(os.environ.get("KRN_WARMUP", "18"))
ROW_PACK = os.environ.get("KRN_ROW_PACK", "1") == "1"

_NC_CACHE = {}


def _build_nc():
    import concourse.mybir as mybir
    import concourse.tile as tile
    from concourse import bacc
    from concourse.masks import make_identity

    F32R = mybir.dt.float32r
    F32 = mybir.dt.float32
    BF = mybir.dt.bfloat16
    MULT = mybir.AluOpType.mult
    ADD = mybir.AluOpType.add

    nc = bacc.Bacc("TRN2", target_bir_lowering=False, debug=False,
                   num_devices=N_CORES)

    q_d = nc.dram_tensor("q", [B, HC, S, D], F32, kind="ExternalInput")
    k_d = nc.dram_tensor("k", [B, HC, S, D], F32, kind="ExternalInput")
    v_d = nc.dram_tensor("v", [B, HC, S, D], F32, kind="ExternalInput")
    om_d = nc.dram_tensor("omask", [HC, S, S], F32, kind="ExternalInput")
    out_d = nc.dram_tensor("out", [B, HC, S, D], F32, kind="ExternalOutput")

    with tile.TileContext(nc) as tc:
        with (
            tc.tile_pool(name="const", bufs=1) as const_pool,
            tc.tile_pool(name="onat", bufs=3) as onat_pool,
            tc.tile_pool(name="omt", bufs=3) as omt_pool,
            tc.tile_pool(name="qkv", bufs=2) as qkv_pool,
            tc.tile_pool(name="work", bufs=3) as work_pool,
            tc.tile_pool(name="small", bufs=4) as small_pool,
            tc.tile_pool(name="outp", bufs=2) as out_pool,
            tc.tile_pool(name="ps_sc", bufs=2, space="PSUM") as ps_sc,
            tc.tile_pool(name="ps_stage", bufs=2, space="PSUM") as ps_stage,
            tc.tile_pool(name="ps_aug", bufs=2, space="PSUM") as ps_aug,
        ):
            # memset/affine_select can't produce f32r directly (and the BIR
            # verifier requires f32r matmul operands to come from an
            # f32r-rounding producer), so build f32 versions and round-copy.
            ident_f = const_pool.tile([P, P], F32, tag="ident_f")
            make_identity(nc, ident_f)
            ident_r = const_pool.tile([P, P], F32R, tag="ident_r")
            nc.vector.tensor_copy(ident_r, ident_f)
            ones_f = const_pool.tile([P, KT], F32, tag="ones_f")
            nc.vector.memset(ones_f, 1.0)

            # PE warmup: ~18 back-to-back matmuls (~7us cold) so the HAM
            # activity monitor lifts the clock gate (1.2 -> 2.4 GHz) before
            # the real matmul stream starts. Overlaps the initial DMA loads.
            warm_w = const_pool.tile([P, P], F32R, tag="warm_w")
            nc.vector.tensor_copy(warm_w, ident_f)
            warm_xf = const_pool.tile([P, QC], F32, tag="warm_xf")
            nc.vector.memset(warm_xf, 1.0)
            warm_x = const_pool.tile([P, QC], F32R, tag="warm_x")
            nc.vector.tensor_copy(warm_x, warm_xf)
            if N_WARMUP:
                warm_ps = ps_aug.tile([P, QC], F32, tag="aug")
                for _ in range(N_WARMUP):
                    nc.tensor.matmul(warm_ps, warm_w, warm_x,
                                     start=True, stop=True)
                warm_sink = small_pool.tile([P, 1], F32, tag="warm_sink")
                nc.vector.tensor_copy(warm_sink, warm_ps[:, 0:1])

            def prep_inputs(b, h):
                # q tiles with the d-column block duplicated ([p, t, 128] =
                # [q | q]), and k tiles packed pairwise ([p, jj, 128] =
                # [block 2jj | block 2jj+1]), rounded to f32r during the
                # SWDGE DMA. A single [128,128] PE transpose of each then
                # directly yields: qT duplicated into both partition halves,
                # and kT pairs with block 2jj on partitions 0-63 / block
                # 2jj+1 on 64-127 — the layout that lets the two K=64 score
                # matmuls of a pair run concurrently in disjoint PE
                # row-groups.
                qsrc = q_d[b, h].rearrange("(t p) d -> p t d", p=P)
                qn2 = qkv_pool.tile([P, QT, P], F32R, tag="qn")
                nc.gpsimd.dma_start(out=qn2[:, :, 0:D], in_=qsrc)
                nc.gpsimd.dma_start(out=qn2[:, :, D:2 * D], in_=qsrc)
                kn2 = qkv_pool.tile([P, KT // 2, 2, D], F32R, tag="kn")
                nc.gpsimd.dma_start(
                    out=kn2,
                    in_=k_d[b, h].rearrange(
                        "(jj two p) d -> p jj two d", p=P, two=2))
                va = qkv_pool.tile([P, KT, D + 1], F32R, tag="va")
                nc.gpsimd.dma_start(
                    out=va[:, :, 0:D],
                    in_=v_d[b, h].rearrange("(t p) d -> p t d", p=P))
                nc.vector.tensor_copy(
                    va[:, :, D:D + 1].rearrange("p t o -> p (t o)"), ones_f)

                qT = qkv_pool.tile([P, S], F32R, tag="qT")
                for g in range(4):
                    stg = ps_stage.tile([P, 4, P], F32R, tag="stage")
                    for i in range(4):
                        nc.tensor.transpose(
                            stg[:, i, :], qn2[:, g * 4 + i, :], ident_r)
                    nc.scalar.copy(
                        out=qT[:, g * 4 * P:(g + 1) * 4 * P]
                            .rearrange("d (i c) -> d i c", c=P),
                        in_=stg)
                if ROW_PACK:
                    kT = qkv_pool.tile([P, KT // 2, P], F32R, tag="kT")
                    for g in range(2):
                        stg = ps_stage.tile([P, 4, P], F32R, tag="stage")
                        for i in range(4):
                            nc.tensor.transpose(
                                stg[:, i, :],
                                kn2[:, g * 4 + i, :, :]
                                    .rearrange("p two d -> p (two d)"),
                                ident_r)
                        nc.scalar.copy(
                            out=kT[:, g * 4:(g + 1) * 4, :], in_=stg)
                else:
                    kT = qkv_pool.tile([D, KT, P], F32R, tag="kT")
                    for g in range(4):
                        stg = ps_stage.tile([D, 4, P], F32R, tag="stage")
                        for i in range(4):
                            j = g * 4 + i
                            nc.tensor.transpose(
                                stg[:, i, :],
                                kn2[:, j // 2, j % 2, :], ident_r)
                        nc.scalar.copy(
                            out=kT[:, g * 4:(g + 1) * 4, :], in_=stg)
                return qT, kT, va

            for h in range(HC):
                msum = small_pool.tile([P, QT], F32, tag="msum")
                rs = small_pool.tile([P, QT], F32, tag="rs")

                prepped = [prep_inputs(b, h) for b in range(B)]

                for ch in range(NCH):
                    csl = slice(ch * TPC, (ch + 1) * TPC)
                    # natural omask tiles for this q-chunk (f32r), 2 halves
                    onats = []
                    for half in range(2):
                        onat = onat_pool.tile([P, 2, S], F32R, tag="onat")
                        r0 = ch * QC + half * 2 * P
                        nc.gpsimd.dma_start(
                            out=onat,
                            in_=om_d[h, r0:r0 + 2 * P, :]
                                .rearrange("(t p) k -> p t k", p=P))
                        onats.append(onat)

                    # msum columns for this chunk's q tiles (accumulate pass
                    # split between DVE and ACT to balance load)
                    for t in range(TPC):
                        qt = ch * TPC + t
                        dummy = work_pool.tile([P, S], BF, tag="msum_dummy")
                        src = onats[t // 2][:, t % 2, :]
                        if t % 2 == 0:
                            nc.vector.tensor_scalar(
                                dummy, src, 1.0, 0.0, MULT, ADD,
                                accum_out=msum[:, qt:qt + 1])
                        else:
                            nc.scalar.activation(
                                dummy, src,
                                mybir.ActivationFunctionType.Copy,
                                accum_out=msum[:, qt:qt + 1])

                    # rs = 1/sqrt(max(msum, tiny)) for this chunk
                    nc.vector.tensor_scalar_max(
                        msum[:, csl], msum[:, csl], 1e-30)
                    nc.scalar.sqrt(rs[:, csl], msum[:, csl])
                    nc.vector.reciprocal(rs[:, csl], rs[:, csl])

                    # build omaskT for this chunk: 2 halves of [128, 8, QC]
                    omts = []
                    for half in range(2):
                        omt = omt_pool.tile([P, KT // 2, QC], F32R, tag="omt")
                        for j8 in range(KT // 2):
                            j = half * (KT // 2) + j8
                            stg = ps_stage.tile([P, TPC, P], F32R, tag="stage")
                            for t in range(TPC):
                                nc.tensor.transpose(
                                    stg[:, t, :],
                                    onats[t // 2][:, t % 2, j * P:(j + 1) * P],
                                    ident_r)
                            nc.scalar.copy(
                                out=omt[:, j8, :]
                                    .rearrange("p (t c) -> p t c", c=P),
                                in_=stg)
                        omts.append(omt)

                    def omt_at(jj):
                        half = (jj * 2) // (KT // 2)
                        j8 = (jj * 2) % (KT // 2)
                        return omts[half][:, j8:j8 + 2, :]

                    for b in range(B):
                        qT, kT, va = prepped[b]
                        aug = ps_aug.tile([D + 1, QC], F32, tag="aug")
                        for jj in range(KT // 2):
                            sc = ps_sc.tile([P, 2, QC], F32, tag="scores")
                            for j2 in range(2):
                                base = j2 * D if ROW_PACK else 0
                                lhsT = (kT[base:base + D, jj, :] if ROW_PACK
                                        else kT[:, jj * 2 + j2, :])
                                nc.tensor.matmul(
                                    sc[:, j2, :], lhsT,
                                    qT[base:base + D,
                                       ch * QC:(ch + 1) * QC],
                                    start=True, stop=True)
                            ret = work_pool.tile([P, 2, QC], F32R, tag="ret")
                            if jj >= GP_PAIRS:
                                # DVE multiply directly from PSUM
                                nc.vector.tensor_mul(ret, sc, omt_at(jj))
                            else:
                                # ACT copy to SBUF + GPSIMD multiply
                                scb = work_pool.tile([P, 2, QC], F32,
                                                     tag="scb")
                                nc.scalar.copy(out=scb, in_=sc)
                                nc.gpsimd.tensor_mul(ret, scb, omt_at(jj))
                            for j2 in range(2):
                                j = jj * 2 + j2
                                nc.tensor.matmul(
                                    aug, va[:, j, :], ret[:, j2, :],
                                    start=(j == 0), stop=(j == KT - 1),
                                    skip_group_check=True)

                        # postprocess: [65, QC] -> scaled [q, d] output
                        augs = work_pool.tile([D + 1, QC], F32, tag="augs")
                        nc.scalar.copy(out=augs, in_=aug)
                        autp = ps_stage.tile([P, TPC, D + 1], F32, tag="stage")
                        for t in range(TPC):
                            nc.tensor.transpose(
                                autp[:, t, :], augs[:, t * P:(t + 1) * P],
                                ident_f[0:D + 1, 0:D + 1])
                        scal = small_pool.tile([P, TPC], F32, tag="scal")
                        nc.vector.tensor_mul(
                            scal,
                            autp[:, :, D:D + 1].rearrange("p t o -> p (t o)"),
                            rs[:, csl])
                        nc.scalar.activation(
                            scal, scal, mybir.ActivationFunctionType.Abs)
                        nc.vector.tensor_scalar_max(scal, scal, 1.0)
                        nc.vector.reciprocal(scal, scal)
                        nc.vector.tensor_mul(scal, scal, rs[:, csl])
                        ob = out_pool.tile([P, TPC, D], F32, tag="ob")
                        for t in range(TPC):
                            nc.vector.tensor_scalar(
                                ob[:, t, :], autp[:, t, 0:D],
                                scal[:, t:t + 1], None, MULT)
                        nc.sync.dma_start(
                            out=out_d[b, h, ch * QC:(ch + 1) * QC, :]
                                .rearrange("(t p) d -> p t d", p=P),
                            in_=ob)

    nc.compile()
    return nc


def _get_nc():
    if "nc" not in _NC_CACHE:
        _NC_CACHE["nc"] = _build_nc()
    return _NC_CACHE["nc"]


def kernel(q, k, v, omask, _trace=False):
    from concourse.bass_utils import run_bass_kernel_spmd

    nc = _get_nc()
    in_maps = []
    for c in range(N_CORES):
        hs = slice(c * HC, (c + 1) * HC)
        in_maps.append({
            "q": np.ascontiguousarray(q[:, hs]),
            "k": np.ascontiguousarray(k[:, hs]),
            "v": np.ascontiguousarray(v[:, hs]),
            "omask": np.ascontiguousarray(omask[hs]),
        })
    res = run_bass_kernel_spmd(nc, in_maps, core_ids=list(range(N_CORES)),
                               trace=_trace)
    out = np.concatenate([res.results[c]["out"] for c in range(N_CORES)],
                         axis=1)
    if _trace:
        kernel.last_results = res
    return out
